# revision 1
# baseline (speedup 1.0000x reference)
"""EuclideanAttention Trainium2 kernel (v6).

Sharding (8 cores = 2 batches x 4 head-groups of 4 heads, Megatron-style
column/row parallel): each core computes, for its (batch b, head group g):
  qT,kT = (x W_{q,k})^T in [e, s] layout (65-partition augmented tiles:
          rows 0-63 data, row 64 ones / -A*|k|^2)
  S^T[j,i] = A*(2 q_i.k_j - |k_j|^2)    (A = 2^7/ln2 folded into the aug
          rows so both exp paths below need no extra scaling work)
  attn^T = exp(S^T/A), split across two engines per j-tile:
    - ACT tiles: native Exp activation (scale=1/A), f32r out
    - DVE tiles: bf16-domain Schraudolph bitcast_i16(max(S^T + B, 0)) --
      one fused tensor_scalar add+max; clamp-to-0 gives exact underflow
  AV + softmax sums via augmented v (ones column -> row 64 = sums); dual
  v copies (f32r + bf16) keep each AV matmul dtype-uniform
  y_partial = vals^T.T @ W_o[row block] in fp16; host sums the 4
  row-parallel partials per batch (the Megatron all-reduce), adds b_o.

Pipeline structure (from sim/HW timeline analysis):
  - projections run sb-major, dt-outer with 4 parallel full-bank PSUM
    accumulation groups, overlapping the interleaved per-d-tile w/x DMA
    stream ([128,512] x tiles: 2KB-run DMAs measured fastest)
  - attention software-pipelines AV two j-tiles behind exp so the
    in-order PE never waits on the ~1.4us exp latency (at pool bufs=4)
  - attention blocks run ih-outer; o_proj tiles for the first half of
    the queries interleave into the second half's blocks (hides y DMA);
    the last block's normalization is chunked to cut its serial latency
  - q/k PSUM->SBUF copies + k2 row scaling + half the y copies run on
    the ACT engine; v copies, normalize, rest of y on DVE; softmax-sum
    broadcast on Pool

Hardware constraints found the hard way:
  - PSUM accumulation groups interleaved across instructions only work
    at full-bank granularity (the N=65 reoriented AV silently corrupts)
  - ACT activation with 2-byte output runs ~2x slower than f32r out
  - gpsimd-initiated DMAs hard-crash the device
  - fp8/bf16 q,k would break the sharp softmax (rel err 0.24/0.06)
"""

import sys

if "/opt/trn_rl_repo" not in sys.path:
    sys.path.insert(0, "/opt/trn_rl_repo")

import numpy as np

import concourse.bacc as bacc
import concourse.mybir as mybir
from concourse.tile import TileContext
from concourse.bass_utils import run_bass_kernel_spmd

F32 = mybir.dt.float32
F32R = mybir.dt.float32r
F16 = mybir.dt.float16
BF16 = mybir.dt.bfloat16
I16 = mybir.dt.int16
I32 = mybir.dt.int32
U16 = mybir.dt.uint16
U32 = mybir.dt.uint32

S = 2048
D = 1024
HD = 64
NH = 4
EC = NH * HD  # 256
ST = S // 128
DT = D // 128
NCORES = 8

A_EXP = float(2.0**7 / np.log(2.0))  # 184.665 (bf16-domain Schraudolph)
B_EXP = 16256.0 - 7.4
# AV reorientation (at stationary, N=65) is numerically broken on HW:
# PSUM accumulation groups interleaved across instructions only work at
# full-bank granularity, and the reoriented AV needs 8 sub-bank groups.
AV_REORIENT = False
# jt tiles assigned to the DVE fast-exp path (rest go to ACT native exp)
DVE_JT = (2, 5, 7, 10, 13, 15)

_CACHED = {}
TRACE = False
LAST_RESULT = None


def build_program(repeat=1):
    nc = bacc.Bacc("TRN2", target_bir_lowering=False, debug=False)
    xt_d = nc.dram_tensor("xt", [D, S], F32R, kind="ExternalInput")
    wq_d = nc.dram_tensor("wq", [D, EC], F32R, kind="ExternalInput")
    wk_d = nc.dram_tensor("wk", [D, EC], F32R, kind="ExternalInput")
    wv_d = nc.dram_tensor("wv", [D, EC], F32R, kind="ExternalInput")
    wo_d = nc.dram_tensor("wo", [EC, D], F32R, kind="ExternalInput")
    y_d = nc.dram_tensor("y", [S, D], F16, kind="ExternalOutput")
    with TileContext(nc) as tc:
        for _ in range(repeat):
            _one_pass(nc, tc, xt_d, wq_d, wk_d, wv_d, wo_d, y_d)
    nc.compile()
    return nc


def _one_pass(nc, tc, xt_d, wq_d, wk_d, wv_d, wo_d, y_d):
    EXP = mybir.ActivationFunctionType.Exp
    CPY = mybir.ActivationFunctionType.Copy
    MUL = mybir.AluOpType.mult
    ADD = mybir.AluOpType.add
    MAX = mybir.AluOpType.max

    with tc.tile_pool(name="persist", bufs=1) as pp:
        qaug = [
            pp.tile([65, S], F32R, tag=f"qaug{h}", name=f"qaug{h}")
            for h in range(NH)
        ]
        kaug = [
            pp.tile([65, S], F32R, tag=f"kaug{h}", name=f"kaug{h}")
            for h in range(NH)
        ]
        # dual v: f32r copy feeds the ACT-exp (f32r at) AV matmuls, bf16
        # copy feeds the DVE-exp (bf16 at) ones — keeps every matmul
        # dtype-uniform (ACT bf16 writes are 2x slow; DVE can't produce f32r)
        vaug = pp.tile([128, ST, NH, HD + 1], F32R, tag="vaug")
        vaug_b = pp.tile([128, ST, NH, HD + 1], BF16, tag="vaugb")
        ones64 = pp.tile([64, 1], F32R, tag="ones64")
        nc.vector.memset(ones64[:].bitcast(U32), 0x3F800000)
        nc.gpsimd.memset(vaug[:, :, :, HD].bitcast(U32), 0x3F800000)
        nc.gpsimd.memset(vaug_b[:, :, :, HD].bitcast(U16), 0x3F80)

        # ---- projections, sb-major so compute pipelines with the x DMA:
        # for each 512-col s-block: q/k proj + v proj + -A*|k|^2 row, while
        # the next s-block's xT tiles stream in.
        with (
            tc.tile_pool(name="xtp", bufs=1) as xp,
            tc.tile_pool(name="wqkv", bufs=1) as wqk,
            tc.tile_pool(name="k2p", bufs=2) as k2p,
            tc.tile_pool(name="psPR", bufs=4, space="PSUM") as psPR,
            tc.tile_pool(name="psVP", bufs=2, space="PSUM") as psVP,
            tc.tile_pool(name="psKS", bufs=2, space="PSUM") as psKS,
        ):
            # DMA order matters: q/k weights + the first s-block of x first
            # (they gate the first matmul), wv before v-proj needs it, the
            # rest of x streams behind the sb loop's compute.
            w_r = {
                nm: wqk.tile([128, DT, EC], F32R, tag=f"w_r{nm}", name=f"wr{nm}")
                for nm in ("q", "k", "v")
            }
            xT = xp.tile([128, DT, S], F32R, tag="xT")

            # w and x interleaved per d-tile, x in [128, 512] s-block tiles
            # (2KB-run DMAs measured markedly faster than 4KB-run half-S
            # tiles); the first q matmul starts after ~0.5MB
            def dma_x_sb(sb):
                for dt_ in range(DT):
                    nc.sync.dma_start(
                        xT[:, dt_, sb * 512 : (sb + 1) * 512],
                        xt_d[
                            dt_ * 128 : (dt_ + 1) * 128,
                            sb * 512 : (sb + 1) * 512,
                        ],
                    )

            for dt_ in range(DT):
                nc.sync.dma_start(
                    w_r["q"][:, dt_, :],
                    wq_d[dt_ * 128 : (dt_ + 1) * 128, :],
                )
                nc.sync.dma_start(
                    xT[:, dt_, 0:512],
                    xt_d[dt_ * 128 : (dt_ + 1) * 128, 0:512],
                )
                nc.sync.dma_start(
                    w_r["k"][:, dt_, :],
                    wk_d[dt_ * 128 : (dt_ + 1) * 128, :],
                )
            nc.sync.dma_start(
                w_r["v"][:], wv_d.rearrange("(dt dl) e -> dl dt e", dl=128)
            )
            for sb in range(1, 4):
                dma_x_sb(sb)

            groups = [
                (nm, dest, scl, et)
                for nm, dest, scl in (("q", qaug, 2.0 * A_EXP), ("k", kaug, 1.0))
                for et in range(EC // 128)
            ]
            for sb in range(4):
                ssl = slice(sb * 512, (sb + 1) * 512)
                # dt-outer with 4 parallel accumulation groups (one full
                # PSUM bank each): every arriving x d-tile is consumed by 4
                # matmuls instead of 1, so the first s-block isn't paced by
                # the x DMA stream
                pss = [
                    psPR.tile([128, 512], F32, tag="projps", name=f"pr{g}")
                    for g in range(4)
                ]
                for dt_ in range(DT):
                    for gi, (nm, dest, scl, et) in enumerate(groups):
                        nc.tensor.matmul(
                            pss[gi][:],
                            w_r[nm][:, dt_, et * 128 : (et + 1) * 128],
                            xT[:, dt_, ssl],
                            start=(dt_ == 0),
                            stop=(dt_ == DT - 1),
                        )
                for gi, (nm, dest, scl, et) in enumerate(groups):
                    for half in range(2):
                        h = et * 2 + half
                        nc.scalar.activation(
                            dest[h][0:64, ssl],
                            pss[gi][half * 64 : (half + 1) * 64, :],
                            CPY,
                            scale=scl,
                        )
                for st in range(sb * 4, sb * 4 + 4):
                    ps = psVP.tile([128, EC], F32, tag="vps")
                    for dt_ in range(DT):
                        nc.tensor.matmul(
                            ps[:],
                            xT[:, dt_, st * 128 : (st + 1) * 128],
                            w_r["v"][:, dt_, :],
                            start=(dt_ == 0),
                            stop=(dt_ == DT - 1),
                        )
                    nc.vector.tensor_copy(
                        out=vaug[:, st, :, 0:HD],
                        in_=ps[:].rearrange("p (h e) -> p h e", h=NH),
                    )
                    nc.vector.tensor_copy(
                        out=vaug_b[:, st, :, 0:HD],
                        in_=ps[:].rearrange("p (h e) -> p h e", h=NH),
                    )
                for h in range(NH):
                    k2 = k2p.tile([64, 512], F32R, tag="k2", name="k2")
                    nc.vector.tensor_tensor(
                        out=k2[:],
                        in0=kaug[h][0:64, ssl],
                        in1=kaug[h][0:64, ssl],
                        op=MUL,
                    )
                    ps = psKS.tile([1, 512], F32, tag="ksps")
                    nc.tensor.matmul(
                        ps[:], ones64[:], k2[:], start=True, stop=True
                    )
                    nc.scalar.activation(
                        kaug[h][64:65, ssl], ps[:], CPY, scale=-A_EXP
                    )
            # ones rows are only read by attention scores; memset them after
            # the projection writes so they don't serialize the startup
            for h in range(NH):
                nc.gpsimd.memset(qaug[h][64:65, :].bitcast(U32), 0x3F800000)

        # ---- attention ----
        with tc.tile_pool(name="latev", bufs=1) as lp:
            valsT = lp.tile([128, EC // 128, S], F32R, tag="valsT")
            wo_r = lp.tile([128, EC // 128, D], F32R, tag="wo_r")
            nc.sync.dma_start(
                wo_r[:], wo_d.rearrange("(et el) f -> el et f", el=128)
            )
            _attn_v3(nc, tc, qaug, kaug, vaug, vaug_b, valsT, wo_r, y_d,
                     EXP, MUL, ADD, MAX, CPY)


def _exp_tile(nc, at, sc, jt, EXP, ADD, MAX):
    """attn^T = exp(sc/A) in bf16; sc holds A*logits.

    Tiles listed in DVE_JT take the DVE fast-exp; the rest use the ACT
    native Exp.
    """
    if jt in DVE_JT:
        nc.vector.tensor_scalar(
            out=at.bitcast(I16),
            in0=sc,
            scalar1=B_EXP,
            scalar2=0.0,
            op0=ADD,
            op1=MAX,
        )
    else:
        nc.scalar.activation(at, sc, EXP, scale=1.0 / A_EXP)


def _attn_v3(nc, tc, qaug, kaug, vaug, vaug_b, valsT, wo_r, y_d,
             EXP, MUL, ADD, MAX, CPY):
    """v3 orientation: vaug stationary, at moving (N=512).

    Blocks run ih-outer so o_proj tiles for the first half of the queries
    interleave with the second half's attention (hiding the y DMA); the
    rest of o_proj runs as the tail. The last block's normalization is
    chunked to shorten its serial latency before the final o tiles.
    """
    with (
        tc.tile_pool(name="normp", bufs=2) as np_,
        tc.tile_pool(name="attnp", bufs=4) as ap_,
        tc.tile_pool(name="attnpb", bufs=4) as apb_,
        tc.tile_pool(name="ysb", bufs=4) as ysb,
        tc.tile_pool(name="scps", bufs=2, space="PSUM") as psS,
        tc.tile_pool(name="avps", bufs=1, space="PSUM") as psAV,
        tc.tile_pool(name="yps", bufs=2, space="PSUM") as psY,
    ):
        yts = {}

        def o_tile(st, db):
            # one full-D f16 row-block per DMA (y rows are contiguous, so
            # 128 descriptors instead of 256 across two half-row DMAs)
            if db == 0:
                yts[st] = ysb.tile([128, D], F16, tag="yt", name=f"yt{st}")
            yt = yts[st]
            ps = psY.tile([128, 512], F32, tag="yps")
            for eb in range(EC // 128):
                nc.tensor.matmul(
                    ps[:],
                    valsT[:, eb, st * 128 : (st + 1) * 128],
                    wo_r[:, eb, db * 512 : (db + 1) * 512],
                    start=(eb == 0),
                    stop=(eb == EC // 128 - 1),
                )
            dsl = slice(db * 512, (db + 1) * 512)
            if db == 0:
                nc.vector.tensor_copy(out=yt[:, dsl], in_=ps[:])
            else:
                nc.scalar.activation(yt[:, dsl], ps[:], CPY)
                nc.sync.dma_start(
                    y_d[st * 128 : (st + 1) * 128, :], yt[:]
                )

        o_pending = []  # (st, db) ready to issue
        for ih in range(2):
            for h in range(NH):
                i0 = ih * 1024
                isl = slice(i0, i0 + 1024)
                av = psAV.tile([HD + 1, 1024], F32, tag="avps")
                DEPTH = 2
                at_q = []

                def issue_av(jv, at_t):
                    vsrc = vaug_b if jv in DVE_JT else vaug
                    for b2 in range(2):
                        nc.tensor.matmul(
                            av[:, b2 * 512 : (b2 + 1) * 512],
                            vsrc[:, jv, h, :],
                            at_t[:, b2 * 512 : (b2 + 1) * 512],
                            start=(jv == 0),
                            stop=(jv == ST - 1),
                        )

                for jt in range(ST):
                    sc = psS.tile([128, 1024], F32, tag="scps")
                    for b2 in range(2):
                        nc.tensor.matmul(
                            sc[:, b2 * 512 : (b2 + 1) * 512],
                            kaug[h][:, jt * 128 : (jt + 1) * 128],
                            qaug[h][:, i0 + b2 * 512 : i0 + (b2 + 1) * 512],
                            start=True,
                            stop=True,
                        )
                    if jt in DVE_JT:
                        at = apb_.tile([128, 1024], BF16, tag="attnb")
                    else:
                        at = ap_.tile([128, 1024], F32R, tag="attn")
                    _exp_tile(nc, at[:], sc[:], jt, EXP, ADD, MAX)
                    if jt % 4 == 3 and o_pending:
                        o_tile(*o_pending.pop(0))
                    at_q.append((jt, at))
                    if len(at_q) > DEPTH:
                        issue_av(*at_q.pop(0))
                for jv, at_t in at_q:
                    issue_av(jv, at_t)
                # normalization; chunked on the final block to cut the
                # serial latency before the o_proj tail
                last = ih == 1 and h == NH - 1
                for csl in ([slice(c * 256, (c + 1) * 256) for c in range(4)]
                            if last else [slice(0, 1024)]):
                    avs = np_.tile([HD + 1, 1024], F32, tag="avs",
                                   name="avs")
                    nc.vector.tensor_copy(out=avs[:, csl], in_=av[:, csl])
                    rec = np_.tile([1, 1024], F32, tag="rec", name="rec")
                    nc.vector.reciprocal(rec[:, csl], avs[HD : HD + 1, csl])
                    rb = np_.tile([64, 1024], F32, tag="rb", name="rb")
                    nc.gpsimd.partition_broadcast(rb[:, csl], rec[:, csl])
                    nc.vector.tensor_tensor(
                        out=valsT[
                            (h % 2) * 64 : (h % 2) * 64 + 64,
                            h // 2,
                            i0 + csl.start : i0 + csl.stop,
                        ],
                        in0=avs[0:HD, csl],
                        in1=rb[:, csl],
                        op=MUL,
                    )
            if ih == 0:
                o_pending = [(st, db) for st in range(8) for db in range(2)]
        for st, db in o_pending:
            o_tile(st, db)
        for st in range(8, ST):
            for db in range(2):
                o_tile(st, db)


def _numpy_fallback(x, W_qkv, b_qkv, W_o, b_o):
    B, S_, D_ = x.shape
    H, Hd = 16, 64
    qkv = x.reshape(-1, D_) @ W_qkv + b_qkv
    qkv = qkv.reshape(B, S_, H, 3 * Hd).transpose(0, 2, 1, 3)
    q, k, v = np.split(qkv, 3, axis=-1)
    out = np.empty((B, S_, D_), np.float32)
    for b in range(B):
        for h in range(H):
            qb, kb, vb = q[b, h], k[b, h], v[b, h]
            lg = 2 * qb @ kb.T - (qb * qb).sum(-1)[:, None] - (kb * kb).sum(-1)[None, :]
            lg -= lg.max(-1, keepdims=True)
            w = np.exp(lg)
            w /= w.sum(-1, keepdims=True)
            out[b, :, h * Hd : (h + 1) * Hd] = w @ vb
    return (out.reshape(-1, D_) @ W_o + b_o).reshape(B, S_, D_)


def make_in_maps(x, W_qkv, W_o):
    Wr = W_qkv.reshape(D, 16, 3, HD)
    xts = [np.ascontiguousarray(x[b].T) for b in range(2)]
    in_maps = []
    for c in range(NCORES):
        b, g = c // 4, c % 4
        e0 = g * EC
        hsl = slice(NH * g, NH * (g + 1))
        in_maps.append(
            {
                "xt": xts[b],
                "wq": np.ascontiguousarray(Wr[:, hsl, 0, :].reshape(D, EC)),
                "wk": np.ascontiguousarray(Wr[:, hsl, 1, :].reshape(D, EC)),
                "wv": np.ascontiguousarray(Wr[:, hsl, 2, :].reshape(D, EC)),
                "wo": np.ascontiguousarray(W_o[e0 : e0 + EC, :]),
            }
        )
    return in_maps


def kernel(x, W_qkv, b_qkv, W_o, b_o):
    x = np.ascontiguousarray(np.asarray(x, dtype=np.float32))
    W_qkv = np.ascontiguousarray(np.asarray(W_qkv, dtype=np.float32))
    b_qkv = np.asarray(b_qkv, dtype=np.float32)
    W_o = np.ascontiguousarray(np.asarray(W_o, dtype=np.float32))
    b_o = np.asarray(b_o, dtype=np.float32)

    if np.any(b_qkv):
        return _numpy_fallback(x, W_qkv, b_qkv, W_o, b_o)

    if "nc" not in _CACHED:
        _CACHED["nc"] = build_program()
    nc = _CACHED["nc"]

    in_maps = make_in_maps(x, W_qkv, W_o)
    kw = {}
    if TRACE:
        kw = dict(trace=True, trace_cores=list(range(NCORES)))
    res = run_bass_kernel_spmd(nc, in_maps, core_ids=list(range(NCORES)), **kw)
    global LAST_RESULT
    LAST_RESULT = res

    out = np.zeros((2, S, D), np.float32)
    for c in range(NCORES):
        out[c // 4] += res.results[c]["y"].astype(np.float32)
    out += b_o
    return out



# revision 24
# speedup vs baseline: 1.0767x; 1.0767x over previous
"""EuclideanAttention Trainium2 kernel (v7).

Sharding (8 cores = 2 batches x 4 head-groups of 4 heads, Megatron-style
column/row parallel): each core computes, for its (batch b, head group g):
  qT,kT = (x W_{q,k})^T in [e, s] layout (65-partition augmented tiles:
          rows 0-63 data, row 64 ones / -A*|k|^2)
  S^T[j,i] = A*(2 q_i.k_j - |k_j|^2)    (A = 2^7/ln2 folded into the aug
          rows so both exp paths below need no extra scaling work)
  attn^T = exp(S^T/A), split across two engines per j-tile:
    - ACT tiles: native Exp activation (scale=1/A), f32r out
    - DVE tiles: bf16-domain Schraudolph bitcast_i16(max(S^T + B, 0)) --
      one fused tensor_scalar add+max; clamp-to-0 gives exact underflow
  AV + softmax sums via augmented v (ones column -> row 64 = sums); the
  AV matmuls run mixed-dtype (f32r v stationary, bf16/f32r at moving)
  y_partial = vals^T.T @ W_o[row block] in fp16; host sums the 4
  row-parallel partials per batch (the Megatron all-reduce), adds b_o.

v11 changes over v6 (sim-timeline + HW-microbench driven):
  - AV accumulates into two single-bank [65,512] PSUM tiles (psAV
    bufs=2) so each half frees right after its normalization reads
  - normalization reads the softmax-sum row STRAIGHT from PSUM (DVE
    reciprocal) in parallel with the av->valsT copy (ACT), then one
    in-place scale (DVE) -- the avs staging buffer and its serial
    copy->recip->bcast->mul chain are gone; av PSUM frees sooner
  - projection q/k PSUM->SBUF copies alternate ACT/DVE so the per-
    s-block copy burst halves (was a ~2.6us PE stall per s-block)
  - o_proj processes a full 128-row query block per unit (both 512-col
    halves, eb-outer); copies split ACT/DVE; tail is 8 units not 32

Hardware constraints found the hard way:
  - matmul moving/free size is capped at 512 (codegen ISA check), so
    everything runs as N=512 instruction pairs; PSUM accumulation
    groups interleaved across instructions only work at full-bank
    granularity (the N=65 reoriented AV silently corrupts)
  - the PE requires both matmul operand dtypes to EQUAL whenever either
    is f32/f32r (walrus verifier) -- no mixed f32r x bf16, hence the
    dual f32r/bf16 v copies feeding the two exp paths
  - TensorTensor needs equal base partitions on its two SBUF inputs
  - ACT activation with 2-byte output runs ~2x slower than f32r out
  - gpsimd-initiated DMAs hard-crash the device
  - fp8/bf16 q,k would break the sharp softmax (rel err 0.24/0.06)
  - HW matmul stationary reloads are ~free (hidden); per-matmul issue
    overhead ~60-150ns is the main HW-vs-cost-model gap
"""

import sys

if "/opt/trn_rl_repo" not in sys.path:
    sys.path.insert(0, "/opt/trn_rl_repo")

import numpy as np

import concourse.bacc as bacc
import concourse.mybir as mybir
from concourse.tile import TileContext
from concourse.bass_utils import run_bass_kernel_spmd

F32 = mybir.dt.float32
F32R = mybir.dt.float32r
F16 = mybir.dt.float16
BF16 = mybir.dt.bfloat16
I16 = mybir.dt.int16
I32 = mybir.dt.int32
U16 = mybir.dt.uint16
U32 = mybir.dt.uint32

S = 2048
D = 1024
HD = 64
NH = 4
EC = NH * HD  # 256
ST = S // 128
DT = D // 128
NCORES = 8

A_EXP = float(2.0**7 / np.log(2.0))  # 184.665 (bf16-domain Schraudolph)
B_EXP = 16256.0 - 7.4
# jt tiles assigned to the DVE fast-exp path (rest go to ACT native exp)
DVE_JT = (2, 5, 7, 10, 13, 15)

_CACHED = {}
TRACE = False
LAST_RESULT = None


def build_program(repeat=1):
    nc = bacc.Bacc("TRN2", target_bir_lowering=False, debug=False)
    xt_d = nc.dram_tensor("xt", [D, S], F32R, kind="ExternalInput")
    wq_d = nc.dram_tensor("wq", [D, EC], F32R, kind="ExternalInput")
    wk_d = nc.dram_tensor("wk", [D, EC], F32R, kind="ExternalInput")
    wv_d = nc.dram_tensor("wv", [D, EC], F32R, kind="ExternalInput")
    wo_d = nc.dram_tensor("wo", [EC, D], F32R, kind="ExternalInput")
    y_d = nc.dram_tensor("y", [S, D], F16, kind="ExternalOutput")
    with TileContext(nc) as tc:
        for _ in range(repeat):
            _one_pass(nc, tc, xt_d, wq_d, wk_d, wv_d, wo_d, y_d)
    nc.compile()
    return nc


def _one_pass(nc, tc, xt_d, wq_d, wk_d, wv_d, wo_d, y_d):
    EXP = mybir.ActivationFunctionType.Exp
    CPY = mybir.ActivationFunctionType.Copy
    MUL = mybir.AluOpType.mult
    ADD = mybir.AluOpType.add
    MAX = mybir.AluOpType.max

    with tc.tile_pool(name="persist", bufs=1) as pp:
        qaug = [
            pp.tile([65, S], F32R, tag=f"qaug{h}", name=f"qaug{h}")
            for h in range(NH)
        ]
        kaug = [
            pp.tile([65, S], F32R, tag=f"kaug{h}", name=f"kaug{h}")
            for h in range(NH)
        ]
        # dual v: f32r copy feeds the ACT-exp (f32r at) AV matmuls, bf16
        # copy feeds the DVE-exp (bf16 at) ones -- the PE requires both
        # operand dtypes to match whenever either is f32/f32r (walrus
        # verifier inst_visitor assert), so mixed-dtype AV is not an option
        vaug = pp.tile([128, ST, NH, HD + 1], F32R, tag="vaug")
        vaug_b = pp.tile([128, ST, NH, HD + 1], BF16, tag="vaugb")
        ones64 = pp.tile([64, 1], F32R, tag="ones64")
        nc.vector.memset(ones64[:].bitcast(U32), 0x3F800000)
        nc.gpsimd.memset(vaug[:, :, :, HD].bitcast(U32), 0x3F800000)
        nc.gpsimd.memset(vaug_b[:, :, :, HD].bitcast(U16), 0x3F80)

        # ---- projections, sb-major so compute pipelines with the x DMA:
        # for each 512-col s-block: q/k proj + v proj + -A*|k|^2 row, while
        # the next s-block's xT tiles stream in.
        with (
            tc.tile_pool(name="xtp", bufs=1) as xp,
            tc.tile_pool(name="wqkv", bufs=1) as wqk,
            tc.tile_pool(name="k2p", bufs=2) as k2p,
            tc.tile_pool(name="psPR", bufs=4, space="PSUM") as psPR,
            tc.tile_pool(name="psVP", bufs=2, space="PSUM") as psVP,
            tc.tile_pool(name="psKS", bufs=2, space="PSUM") as psKS,
        ):
            # DMA order matters: q/k weights + the first s-block of x first
            # (they gate the first matmul), wv before v-proj needs it, the
            # rest of x streams behind the sb loop's compute.
            w_r = {
                nm: wqk.tile([128, DT, EC], F32R, tag=f"w_r{nm}", name=f"wr{nm}")
                for nm in ("q", "k", "v")
            }
            xT = xp.tile([128, DT, S], F32R, tag="xT")

            # w and x interleaved per d-tile, x in [128, 512] s-block tiles
            # (2KB-run DMAs measured markedly faster than 4KB-run half-S
            # tiles); the first q matmul starts after ~0.5MB
            def dma_x_sb(sb):
                for dt_ in range(DT):
                    nc.sync.dma_start(
                        xT[:, dt_, sb * 512 : (sb + 1) * 512],
                        xt_d[
                            dt_ * 128 : (dt_ + 1) * 128,
                            sb * 512 : (sb + 1) * 512,
                        ],
                    )

            for dt_ in range(DT):
                nc.sync.dma_start(
                    w_r["q"][:, dt_, :],
                    wq_d[dt_ * 128 : (dt_ + 1) * 128, :],
                )
                nc.sync.dma_start(
                    xT[:, dt_, 0:512],
                    xt_d[dt_ * 128 : (dt_ + 1) * 128, 0:512],
                )
                nc.sync.dma_start(
                    w_r["k"][:, dt_, :],
                    wk_d[dt_ * 128 : (dt_ + 1) * 128, :],
                )
            nc.sync.dma_start(
                w_r["v"][:], wv_d.rearrange("(dt dl) e -> dl dt e", dl=128)
            )
            for sb in range(1, 4):
                dma_x_sb(sb)

            groups = [
                (nm, dest, scl, et)
                for nm, dest, scl in (("q", qaug, 2.0 * A_EXP), ("k", kaug, 1.0))
                for et in range(EC // 128)
            ]
            for sb in range(4):
                ssl = slice(sb * 512, (sb + 1) * 512)
                # dt-outer with 4 parallel accumulation groups (one full
                # PSUM bank each): every arriving x d-tile is consumed by 4
                # matmuls instead of 1, so the first s-block isn't paced by
                # the x DMA stream
                pss = [
                    psPR.tile([128, 512], F32, tag="projps", name=f"pr{g}")
                    for g in range(4)
                ]
                for dt_ in range(DT):
                    for gi, (nm, dest, scl, et) in enumerate(groups):
                        nc.tensor.matmul(
                            pss[gi][:],
                            w_r[nm][:, dt_, et * 128 : (et + 1) * 128],
                            xT[:, dt_, ssl],
                            start=(dt_ == 0),
                            stop=(dt_ == DT - 1),
                        )
                # copy-out burst alternates ACT/DVE so neither engine gates
                # the next s-block's matmul group by itself
                for gi, (nm, dest, scl, et) in enumerate(groups):
                    for half in range(2):
                        h = et * 2 + half
                        src = pss[gi][half * 64 : (half + 1) * 64, :]
                        if (gi * 2 + half) % 2 == 0:
                            nc.scalar.activation(
                                dest[h][0:64, ssl], src, CPY, scale=scl
                            )
                        elif scl == 1.0:
                            nc.vector.tensor_copy(
                                out=dest[h][0:64, ssl], in_=src
                            )
                        else:
                            nc.vector.tensor_scalar_mul(
                                out=dest[h][0:64, ssl], in0=src, scalar1=scl
                            )
                for st in range(sb * 4, sb * 4 + 4):
                    ps = psVP.tile([128, EC], F32, tag="vps")
                    for dt_ in range(DT):
                        nc.tensor.matmul(
                            ps[:],
                            xT[:, dt_, st * 128 : (st + 1) * 128],
                            w_r["v"][:, dt_, :],
                            start=(dt_ == 0),
                            stop=(dt_ == DT - 1),
                        )
                    nc.vector.tensor_copy(
                        out=vaug[:, st, :, 0:HD],
                        in_=ps[:].rearrange("p (h e) -> p h e", h=NH),
                    )
                    nc.vector.tensor_copy(
                        out=vaug_b[:, st, :, 0:HD],
                        in_=ps[:].rearrange("p (h e) -> p h e", h=NH),
                    )
                for h in range(NH):
                    k2 = k2p.tile([64, 512], F32R, tag="k2", name="k2")
                    nc.vector.tensor_tensor(
                        out=k2[:],
                        in0=kaug[h][0:64, ssl],
                        in1=kaug[h][0:64, ssl],
                        op=MUL,
                    )
                    ps = psKS.tile([1, 512], F32, tag="ksps")
                    nc.tensor.matmul(
                        ps[:], ones64[:], k2[:], start=True, stop=True
                    )
                    nc.scalar.activation(
                        kaug[h][64:65, ssl], ps[:], CPY, scale=-A_EXP
                    )
            # ones rows are only read by attention scores; memset them after
            # the projection writes so they don't serialize the startup
            for h in range(NH):
                nc.gpsimd.memset(qaug[h][64:65, :].bitcast(U32), 0x3F800000)

        # ---- attention ----
        with tc.tile_pool(name="latev", bufs=1) as lp:
            valsT = lp.tile([128, EC // 128, S], F32R, tag="valsT")
            wo_r = lp.tile([128, EC // 128, D], F32R, tag="wo_r")
            nc.sync.dma_start(
                wo_r[:], wo_d.rearrange("(et el) f -> el et f", el=128)
            )
            _attn_v4(nc, tc, qaug, kaug, vaug, vaug_b, valsT, wo_r, y_d,
                     EXP, MUL, ADD, MAX, CPY)


def _exp_tile(nc, at, sc, jt, EXP, ADD, MAX):
    """attn^T = exp(sc/A) in bf16; sc holds A*logits.

    Tiles listed in DVE_JT take the DVE fast-exp; the rest use the ACT
    native Exp.
    """
    if jt in DVE_JT:
        nc.vector.tensor_scalar(
            out=at.bitcast(I16),
            in0=sc,
            scalar1=B_EXP,
            scalar2=0.0,
            op0=ADD,
            op1=MAX,
        )
    else:
        nc.scalar.activation(at, sc, EXP, scale=1.0 / A_EXP)


def _attn_v4(nc, tc, qaug, kaug, vaug, vaug_b, valsT, wo_r, y_d,
             EXP, MUL, ADD, MAX, CPY):
    """v4 orientation: vaug stationary, at moving (N=512).

    Blocks run ih-outer so o_proj units for the first half of the queries
    interleave with the second half's attention (hiding the y DMA). Each
    block's AV accumulates into two single-bank [65,512] PSUM tiles that
    free independently right after their normalization reads.
    """
    with (
        tc.tile_pool(name="normp", bufs=2) as np_,
        tc.tile_pool(name="attnp", bufs=4) as ap_,
        tc.tile_pool(name="attnpb", bufs=4) as apb_,
        tc.tile_pool(name="ysb", bufs=4) as ysb,
        tc.tile_pool(name="scps", bufs=2, space="PSUM") as psS,
        tc.tile_pool(name="avps", bufs=2, space="PSUM") as psAV,
        tc.tile_pool(name="yps", bufs=2, space="PSUM") as psY,
    ):
        def o_st(st):
            # both 512-col halves of one 128-query row block, eb-outer so
            # each valsT stationary serves two matmuls; copies split
            # ACT/DVE; one full-D f16 row-block per DMA
            yt = ysb.tile([128, D], F16, tag="yt", name=f"yt{st}")
            ps = [
                psY.tile([128, 512], F32, tag="yps", name=f"yps{db}")
                for db in range(2)
            ]
            for eb in range(EC // 128):
                for db in range(2):
                    nc.tensor.matmul(
                        ps[db][:],
                        valsT[:, eb, st * 128 : (st + 1) * 128],
                        wo_r[:, eb, db * 512 : (db + 1) * 512],
                        start=(eb == 0),
                        stop=(eb == EC // 128 - 1),
                    )
            nc.vector.tensor_copy(out=yt[:, 0:512], in_=ps[0][:])
            nc.scalar.activation(yt[:, 512:1024], ps[1][:], CPY)
            nc.sync.dma_start(y_d[st * 128 : (st + 1) * 128, :], yt[:])

        o_pending = []  # sts ready to issue
        for ih in range(2):
            for h in range(NH):
                i0 = ih * 1024
                # AV accumulates into two single-bank [65,512] tiles that
                # free independently right after their normalization reads
                av = [
                    psAV.tile([HD + 1, 512], F32, tag="avps", name=f"av{b2}")
                    for b2 in range(2)
                ]
                DEPTH = 2
                at_q = []

                def issue_av(jv, at_t):
                    vsrc = vaug_b if jv in DVE_JT else vaug
                    for b2 in range(2):
                        nc.tensor.matmul(
                            av[b2][:],
                            vsrc[:, jv, h, :],
                            at_t[:, b2 * 512 : (b2 + 1) * 512],
                            start=(jv == 0),
                            stop=(jv == ST - 1),
                        )

                for jt in range(ST):
                    sc = psS.tile([128, 1024], F32, tag="scps")
                    for b2 in range(2):
                        nc.tensor.matmul(
                            sc[:, b2 * 512 : (b2 + 1) * 512],
                            kaug[h][:, jt * 128 : (jt + 1) * 128],
                            qaug[h][:, i0 + b2 * 512 : i0 + (b2 + 1) * 512],
                            start=True,
                            stop=True,
                        )
                    if jt in DVE_JT:
                        at = apb_.tile([128, 1024], BF16, tag="attnb")
                    else:
                        at = ap_.tile([128, 1024], F32R, tag="attn")
                    _exp_tile(nc, at[:], sc[:], jt, EXP, ADD, MAX)
                    if jt % 8 == 3 and o_pending:
                        o_st(o_pending.pop(0))
                    at_q.append((jt, at))
                    if len(at_q) > DEPTH:
                        issue_av(*at_q.pop(0))
                for jv, at_t in at_q:
                    issue_av(jv, at_t)
                # per-half normalization: reciprocal reads the sums row
                # straight from PSUM (DVE), broadcast on Pool, then one
                # DVE multiply reads the values straight from PSUM and
                # writes normalized valsT -- no staging copy at all
                p0 = (h % 2) * 64
                for b2 in range(2):
                    rec = np_.tile([1, 512], F32, tag="rec", name="rec")
                    nc.vector.reciprocal(rec[:], av[b2][HD : HD + 1, :])
                    rb = np_.tile([64, 512], F32, tag="rb", name="rb")
                    nc.gpsimd.partition_broadcast(rb[:], rec[:])
                    dst = valsT[
                        p0 : p0 + 64,
                        h // 2,
                        i0 + b2 * 512 : i0 + (b2 + 1) * 512,
                    ]
                    nc.vector.tensor_tensor(
                        out=dst, in0=av[b2][0:HD, :], in1=rb[:], op=MUL
                    )
            if ih == 0:
                o_pending = list(range(8))
        for st in o_pending:
            o_st(st)
        for st in range(8, ST):
            o_st(st)


def _numpy_fallback(x, W_qkv, b_qkv, W_o, b_o):
    B, S_, D_ = x.shape
    H, Hd = 16, 64
    qkv = x.reshape(-1, D_) @ W_qkv + b_qkv
    qkv = qkv.reshape(B, S_, H, 3 * Hd).transpose(0, 2, 1, 3)
    q, k, v = np.split(qkv, 3, axis=-1)
    out = np.empty((B, S_, D_), np.float32)
    for b in range(B):
        for h in range(H):
            qb, kb, vb = q[b, h], k[b, h], v[b, h]
            lg = 2 * qb @ kb.T - (qb * qb).sum(-1)[:, None] - (kb * kb).sum(-1)[None, :]
            lg -= lg.max(-1, keepdims=True)
            w = np.exp(lg)
            w /= w.sum(-1, keepdims=True)
            out[b, :, h * Hd : (h + 1) * Hd] = w @ vb
    return (out.reshape(-1, D_) @ W_o + b_o).reshape(B, S_, D_)


def make_in_maps(x, W_qkv, W_o):
    Wr = W_qkv.reshape(D, 16, 3, HD)
    xts = [np.ascontiguousarray(x[b].T) for b in range(2)]
    in_maps = []
    for c in range(NCORES):
        b, g = c // 4, c % 4
        e0 = g * EC
        hsl = slice(NH * g, NH * (g + 1))
        in_maps.append(
            {
                "xt": xts[b],
                "wq": np.ascontiguousarray(Wr[:, hsl, 0, :].reshape(D, EC)),
                "wk": np.ascontiguousarray(Wr[:, hsl, 1, :].reshape(D, EC)),
                "wv": np.ascontiguousarray(Wr[:, hsl, 2, :].reshape(D, EC)),
                "wo": np.ascontiguousarray(W_o[e0 : e0 + EC, :]),
            }
        )
    return in_maps


def kernel(x, W_qkv, b_qkv, W_o, b_o):
    x = np.ascontiguousarray(np.asarray(x, dtype=np.float32))
    W_qkv = np.ascontiguousarray(np.asarray(W_qkv, dtype=np.float32))
    b_qkv = np.asarray(b_qkv, dtype=np.float32)
    W_o = np.ascontiguousarray(np.asarray(W_o, dtype=np.float32))
    b_o = np.asarray(b_o, dtype=np.float32)

    if np.any(b_qkv):
        return _numpy_fallback(x, W_qkv, b_qkv, W_o, b_o)

    if "nc" not in _CACHED:
        _CACHED["nc"] = build_program()
    nc = _CACHED["nc"]

    in_maps = make_in_maps(x, W_qkv, W_o)
    kw = {}
    if TRACE:
        kw = dict(trace=True, trace_cores=list(range(NCORES)))
    res = run_bass_kernel_spmd(nc, in_maps, core_ids=list(range(NCORES)), **kw)
    global LAST_RESULT
    LAST_RESULT = res

    out = np.zeros((2, S, D), np.float32)
    for c in range(NCORES):
        out[c // 4] += res.results[c]["y"].astype(np.float32)
    out += b_o
    return out


# revision 29
# speedup vs baseline: 1.1052x; 1.0264x over previous
"""EuclideanAttention Trainium2 kernel (v7).

Sharding (8 cores = 2 batches x 4 head-groups of 4 heads, Megatron-style
column/row parallel): each core computes, for its (batch b, head group g):
  qT,kT = (x W_{q,k})^T in [e, s] layout (65-partition augmented tiles:
          rows 0-63 data, row 64 ones / -A*|k|^2)
  S^T[j,i] = A*(2 q_i.k_j - |k_j|^2)    (A = 2^7/ln2 folded into the aug
          rows so both exp paths below need no extra scaling work)
  attn^T = exp(S^T/A), split across two engines per j-tile:
    - ACT tiles: native Exp activation (scale=1/A), f32r out
    - DVE tiles: bf16-domain Schraudolph bitcast_i16(max(S^T + B, 0)) --
      one fused tensor_scalar add+max; clamp-to-0 gives exact underflow
  AV + softmax sums via augmented v (ones column -> row 64 = sums); the
  AV matmuls run mixed-dtype (f32r v stationary, bf16/f32r at moving)
  y_partial = vals^T.T @ W_o[row block] in fp16; host sums the 4
  row-parallel partials per batch (the Megatron all-reduce), adds b_o.

v11 changes over v6 (sim-timeline + HW-microbench driven):
  - AV accumulates into two single-bank [65,512] PSUM tiles (psAV
    bufs=2) so each half frees right after its normalization reads
  - normalization reads the softmax-sum row STRAIGHT from PSUM (DVE
    reciprocal) in parallel with the av->valsT copy (ACT), then one
    in-place scale (DVE) -- the avs staging buffer and its serial
    copy->recip->bcast->mul chain are gone; av PSUM frees sooner
  - projection q/k PSUM->SBUF copies alternate ACT/DVE so the per-
    s-block copy burst halves (was a ~2.6us PE stall per s-block)
  - o_proj processes a full 128-row query block per unit (both 512-col
    halves, eb-outer); copies split ACT/DVE; tail is 8 units not 32

Hardware constraints found the hard way:
  - matmul moving/free size is capped at 512 (codegen ISA check), so
    everything runs as N=512 instruction pairs; PSUM accumulation
    groups interleaved across instructions only work at full-bank
    granularity (the N=65 reoriented AV silently corrupts)
  - the PE requires both matmul operand dtypes to EQUAL whenever either
    is f32/f32r (walrus verifier) -- no mixed f32r x bf16, hence the
    dual f32r/bf16 v copies feeding the two exp paths
  - TensorTensor needs equal base partitions on its two SBUF inputs
  - ACT activation with 2-byte output runs ~2x slower than f32r out
  - gpsimd-initiated DMAs hard-crash the device
  - fp8/bf16 q,k would break the sharp softmax (rel err 0.24/0.06)
  - HW matmul stationary reloads are ~free (hidden); per-matmul issue
    overhead ~60-150ns is the main HW-vs-cost-model gap
"""

import sys

if "/opt/trn_rl_repo" not in sys.path:
    sys.path.insert(0, "/opt/trn_rl_repo")

import numpy as np

import concourse.bacc as bacc
import concourse.mybir as mybir
from concourse.tile import TileContext
from concourse.bass_utils import run_bass_kernel_spmd

F32 = mybir.dt.float32
F32R = mybir.dt.float32r
F16 = mybir.dt.float16
BF16 = mybir.dt.bfloat16
I16 = mybir.dt.int16
I32 = mybir.dt.int32
U16 = mybir.dt.uint16
U32 = mybir.dt.uint32

S = 2048
D = 1024
HD = 64
NH = 4
EC = NH * HD  # 256
ST = S // 128
DT = D // 128
NCORES = 8

A_EXP = float(2.0**7 / np.log(2.0))  # 184.665 (bf16-domain Schraudolph)
B_EXP = 16256.0 - 7.4
# jt tiles assigned to the DVE fast-exp path (rest go to ACT native exp)
DVE_JT = (2, 5, 7, 10, 13, 15)

_CACHED = {}
TRACE = False
LAST_RESULT = None


def build_program(repeat=1):
    nc = bacc.Bacc("TRN2", target_bir_lowering=False, debug=False)
    xt_d = nc.dram_tensor("xt", [D, S], F32R, kind="ExternalInput")
    wq_d = nc.dram_tensor("wq", [D, EC], F32R, kind="ExternalInput")
    wk_d = nc.dram_tensor("wk", [D, EC], F32R, kind="ExternalInput")
    wv_d = nc.dram_tensor("wv", [D, EC], F32R, kind="ExternalInput")
    wo_d = nc.dram_tensor("wo", [EC, D], F32R, kind="ExternalInput")
    y_d = nc.dram_tensor("y", [S, D], F16, kind="ExternalOutput")
    with TileContext(nc) as tc:
        for _ in range(repeat):
            _one_pass(nc, tc, xt_d, wq_d, wk_d, wv_d, wo_d, y_d)
    nc.compile()
    return nc


def _one_pass(nc, tc, xt_d, wq_d, wk_d, wv_d, wo_d, y_d):
    EXP = mybir.ActivationFunctionType.Exp
    CPY = mybir.ActivationFunctionType.Copy
    MUL = mybir.AluOpType.mult
    ADD = mybir.AluOpType.add
    MAX = mybir.AluOpType.max

    with tc.tile_pool(name="persist", bufs=1) as pp:
        qaug = [
            pp.tile([65, S], F32R, tag=f"qaug{h}", name=f"qaug{h}")
            for h in range(NH)
        ]
        kaug = [
            pp.tile([65, S], F32R, tag=f"kaug{h}", name=f"kaug{h}")
            for h in range(NH)
        ]
        # dual v: f32r copy feeds the ACT-exp (f32r at) AV matmuls, bf16
        # copy feeds the DVE-exp (bf16 at) ones -- the PE requires both
        # operand dtypes to match whenever either is f32/f32r (walrus
        # verifier inst_visitor assert), so mixed-dtype AV is not an option
        vaug = pp.tile([128, ST, NH, HD + 1], F32R, tag="vaug")
        vaug_b = pp.tile([128, ST, NH, HD + 1], BF16, tag="vaugb")
        ones64 = pp.tile([64, 1], F32R, tag="ones64")
        nc.vector.memset(ones64[:].bitcast(U32), 0x3F800000)
        nc.gpsimd.memset(vaug[:, :, :, HD].bitcast(U32), 0x3F800000)
        nc.gpsimd.memset(vaug_b[:, :, :, HD].bitcast(U16), 0x3F80)

        # ---- projections, sb-major so compute pipelines with the x DMA:
        # for each 512-col s-block: q/k proj + v proj + -A*|k|^2 row, while
        # the next s-block's xT tiles stream in.
        with (
            tc.tile_pool(name="xtp", bufs=1) as xp,
            tc.tile_pool(name="wqkv", bufs=1) as wqk,
            tc.tile_pool(name="k2p", bufs=2) as k2p,
            tc.tile_pool(name="psPR", bufs=4, space="PSUM") as psPR,
            tc.tile_pool(name="psVP", bufs=2, space="PSUM") as psVP,
            tc.tile_pool(name="psKS", bufs=2, space="PSUM") as psKS,
        ):
            # DMA order matters: q/k weights + the first s-block of x first
            # (they gate the first matmul), wv before v-proj needs it, the
            # rest of x streams behind the sb loop's compute.
            w_r = {
                nm: wqk.tile([128, DT, EC], F32R, tag=f"w_r{nm}", name=f"wr{nm}")
                for nm in ("q", "k", "v")
            }
            xT = xp.tile([128, DT, S], F32R, tag="xT")

            # w and x interleaved per d-tile, x in [128, 512] s-block tiles
            # (2KB-run DMAs measured markedly faster than 4KB-run half-S
            # tiles); the first q matmul starts after ~0.5MB
            # HWDGE descriptor generation is ~625ns per DMA, serialized per
            # issuing queue -- split the x stream so sb2/sb3 generate on the
            # ACT queue in parallel with SP's w + sb0/sb1 stream (ACT's
            # generation burst drains before its first copy-out is due)
            def dma_x_sb(sb):
                eng = nc.sync if sb < 2 else nc.scalar
                for dt_ in range(DT):
                    eng.dma_start(
                        xT[:, dt_, sb * 512 : (sb + 1) * 512],
                        xt_d[
                            dt_ * 128 : (dt_ + 1) * 128,
                            sb * 512 : (sb + 1) * 512,
                        ],
                    )

            for dt_ in range(DT):
                nc.sync.dma_start(
                    w_r["q"][:, dt_, :],
                    wq_d[dt_ * 128 : (dt_ + 1) * 128, :],
                )
                nc.sync.dma_start(
                    xT[:, dt_, 0:512],
                    xt_d[dt_ * 128 : (dt_ + 1) * 128, 0:512],
                )
                nc.sync.dma_start(
                    w_r["k"][:, dt_, :],
                    wk_d[dt_ * 128 : (dt_ + 1) * 128, :],
                )
            nc.sync.dma_start(
                w_r["v"][:], wv_d.rearrange("(dt dl) e -> dl dt e", dl=128)
            )
            for sb in range(1, 4):
                dma_x_sb(sb)

            groups = [
                (nm, dest, scl, et)
                for nm, dest, scl in (("q", qaug, 2.0 * A_EXP), ("k", kaug, 1.0))
                for et in range(EC // 128)
            ]
            for sb in range(4):
                ssl = slice(sb * 512, (sb + 1) * 512)
                # dt-outer with 4 parallel accumulation groups (one full
                # PSUM bank each): every arriving x d-tile is consumed by 4
                # matmuls instead of 1, so the first s-block isn't paced by
                # the x DMA stream
                pss = [
                    psPR.tile([128, 512], F32, tag="projps", name=f"pr{g}")
                    for g in range(4)
                ]
                for dt_ in range(DT):
                    for gi, (nm, dest, scl, et) in enumerate(groups):
                        nc.tensor.matmul(
                            pss[gi][:],
                            w_r[nm][:, dt_, et * 128 : (et + 1) * 128],
                            xT[:, dt_, ssl],
                            start=(dt_ == 0),
                            stop=(dt_ == DT - 1),
                        )
                # copy-out burst alternates ACT/DVE so neither engine gates
                # the next s-block's matmul group by itself
                for gi, (nm, dest, scl, et) in enumerate(groups):
                    for half in range(2):
                        h = et * 2 + half
                        src = pss[gi][half * 64 : (half + 1) * 64, :]
                        if (gi * 2 + half) % 2 == 0:
                            nc.scalar.activation(
                                dest[h][0:64, ssl], src, CPY, scale=scl
                            )
                        elif scl == 1.0:
                            nc.vector.tensor_copy(
                                out=dest[h][0:64, ssl], in_=src
                            )
                        else:
                            nc.vector.tensor_scalar_mul(
                                out=dest[h][0:64, ssl], in0=src, scalar1=scl
                            )
                for st in range(sb * 4, sb * 4 + 4):
                    ps = psVP.tile([128, EC], F32, tag="vps")
                    for dt_ in range(DT):
                        nc.tensor.matmul(
                            ps[:],
                            xT[:, dt_, st * 128 : (st + 1) * 128],
                            w_r["v"][:, dt_, :],
                            start=(dt_ == 0),
                            stop=(dt_ == DT - 1),
                        )
                    nc.vector.tensor_copy(
                        out=vaug[:, st, :, 0:HD],
                        in_=ps[:].rearrange("p (h e) -> p h e", h=NH),
                    )
                    nc.vector.tensor_copy(
                        out=vaug_b[:, st, :, 0:HD],
                        in_=ps[:].rearrange("p (h e) -> p h e", h=NH),
                    )
                for h in range(NH):
                    k2 = k2p.tile([64, 512], F32R, tag="k2", name="k2")
                    nc.vector.tensor_tensor(
                        out=k2[:],
                        in0=kaug[h][0:64, ssl],
                        in1=kaug[h][0:64, ssl],
                        op=MUL,
                    )
                    ps = psKS.tile([1, 512], F32, tag="ksps")
                    nc.tensor.matmul(
                        ps[:], ones64[:], k2[:], start=True, stop=True
                    )
                    nc.scalar.activation(
                        kaug[h][64:65, ssl], ps[:], CPY, scale=-A_EXP
                    )
            # ones rows are only read by attention scores; memset them after
            # the projection writes so they don't serialize the startup
            for h in range(NH):
                nc.gpsimd.memset(qaug[h][64:65, :].bitcast(U32), 0x3F800000)

        # ---- attention ----
        with tc.tile_pool(name="latev", bufs=1) as lp:
            valsT = lp.tile([128, EC // 128, S], F32R, tag="valsT")
            wo_r = lp.tile([128, EC // 128, D], F32R, tag="wo_r")
            nc.sync.dma_start(
                wo_r[:], wo_d.rearrange("(et el) f -> el et f", el=128)
            )
            _attn_v4(nc, tc, qaug, kaug, vaug, vaug_b, valsT, wo_r, y_d,
                     EXP, MUL, ADD, MAX, CPY)


def _exp_tile(nc, at, sc, jt, EXP, ADD, MAX):
    """attn^T = exp(sc/A) in bf16; sc holds A*logits.

    Tiles listed in DVE_JT take the DVE fast-exp; the rest use the ACT
    native Exp.
    """
    if jt in DVE_JT:
        nc.vector.tensor_scalar(
            out=at.bitcast(I16),
            in0=sc,
            scalar1=B_EXP,
            scalar2=0.0,
            op0=ADD,
            op1=MAX,
        )
    else:
        nc.scalar.activation(at, sc, EXP, scale=1.0 / A_EXP)


def _attn_v4(nc, tc, qaug, kaug, vaug, vaug_b, valsT, wo_r, y_d,
             EXP, MUL, ADD, MAX, CPY):
    """v4 orientation: vaug stationary, at moving (N=512).

    Blocks run ih-outer so o_proj units for the first half of the queries
    interleave with the second half's attention (hiding the y DMA). Each
    block's AV accumulates into two single-bank [65,512] PSUM tiles that
    free independently right after their normalization reads.
    """
    with (
        tc.tile_pool(name="normp", bufs=2) as np_,
        tc.tile_pool(name="attnp", bufs=4) as ap_,
        tc.tile_pool(name="attnpb", bufs=4) as apb_,
        tc.tile_pool(name="ysb", bufs=4) as ysb,
        tc.tile_pool(name="scps", bufs=2, space="PSUM") as psS,
        tc.tile_pool(name="avps", bufs=2, space="PSUM") as psAV,
        tc.tile_pool(name="yps", bufs=2, space="PSUM") as psY,
    ):
        def o_st(st):
            # both 512-col halves of one 128-query row block, eb-outer so
            # each valsT stationary serves two matmuls; copies split
            # ACT/DVE; one full-D f16 row-block per DMA
            yt = ysb.tile([128, D], F16, tag="yt", name=f"yt{st}")
            ps = [
                psY.tile([128, 512], F32, tag="yps", name=f"yps{db}")
                for db in range(2)
            ]
            for eb in range(EC // 128):
                for db in range(2):
                    nc.tensor.matmul(
                        ps[db][:],
                        valsT[:, eb, st * 128 : (st + 1) * 128],
                        wo_r[:, eb, db * 512 : (db + 1) * 512],
                        start=(eb == 0),
                        stop=(eb == EC // 128 - 1),
                    )
            # both copies on DVE (2x throughput on 16-bit writes); the y
            # DMA goes out the ACT queue -- keeping y off the SP queue
            # lets the next pass's input DMAs issue early (the SP queue
            # otherwise stalls behind y triggers waiting on late compute,
            # serializing cross-pass prefetch). Only SP/ACT may initiate
            # DMAs (DVE can't; gpsimd hard-crashes).
            nc.vector.tensor_copy(out=yt[:, 0:512], in_=ps[0][:])
            nc.vector.tensor_copy(out=yt[:, 512:1024], in_=ps[1][:])
            nc.scalar.dma_start(y_d[st * 128 : (st + 1) * 128, :], yt[:])

        o_pending = []  # sts ready to issue
        for ih in range(2):
            for h in range(NH):
                i0 = ih * 1024
                # AV accumulates into two single-bank [65,512] tiles that
                # free independently right after their normalization reads
                av = [
                    psAV.tile([HD + 1, 512], F32, tag="avps", name=f"av{b2}")
                    for b2 in range(2)
                ]
                DEPTH = 3
                at_q = []

                def issue_av(jv, at_t):
                    vsrc = vaug_b if jv in DVE_JT else vaug
                    for b2 in range(2):
                        nc.tensor.matmul(
                            av[b2][:],
                            vsrc[:, jv, h, :],
                            at_t[:, b2 * 512 : (b2 + 1) * 512],
                            start=(jv == 0),
                            stop=(jv == ST - 1),
                        )

                for jt in range(ST):
                    sc = psS.tile([128, 1024], F32, tag="scps")
                    for b2 in range(2):
                        nc.tensor.matmul(
                            sc[:, b2 * 512 : (b2 + 1) * 512],
                            kaug[h][:, jt * 128 : (jt + 1) * 128],
                            qaug[h][:, i0 + b2 * 512 : i0 + (b2 + 1) * 512],
                            start=True,
                            stop=True,
                        )
                    if jt in DVE_JT:
                        at = apb_.tile([128, 1024], BF16, tag="attnb")
                    else:
                        at = ap_.tile([128, 1024], F32R, tag="attn")
                    _exp_tile(nc, at[:], sc[:], jt, EXP, ADD, MAX)
                    if jt % 8 == 3 and o_pending:
                        o_st(o_pending.pop(0))
                    at_q.append((jt, at))
                    if len(at_q) > DEPTH:
                        issue_av(*at_q.pop(0))
                for jv, at_t in at_q:
                    issue_av(jv, at_t)
                # per-half normalization: reciprocal reads the sums row
                # straight from PSUM (DVE) in parallel with the ACT copy
                # of the values to a partition-0 staging tile -- each av
                # bank frees after those two reads, before the bcast+mul
                p0 = (h % 2) * 64
                for b2 in range(2):
                    rec = np_.tile([1, 512], F32, tag="rec", name="rec")
                    nc.vector.reciprocal(rec[:], av[b2][HD : HD + 1, :])
                    avs = np_.tile([64, 512], F32, tag="avs", name="avs")
                    nc.scalar.activation(avs[:], av[b2][0:HD, :], CPY)
                    rb = np_.tile([64, 512], F32, tag="rb", name="rb")
                    nc.gpsimd.partition_broadcast(rb[:], rec[:])
                    dst = valsT[
                        p0 : p0 + 64,
                        h // 2,
                        i0 + b2 * 512 : i0 + (b2 + 1) * 512,
                    ]
                    nc.vector.tensor_tensor(
                        out=dst, in0=avs[:], in1=rb[:], op=MUL
                    )
            if ih == 0:
                o_pending = list(range(8))
        for st in o_pending:
            o_st(st)
        for st in range(8, ST):
            o_st(st)


def _numpy_fallback(x, W_qkv, b_qkv, W_o, b_o):
    B, S_, D_ = x.shape
    H, Hd = 16, 64
    qkv = x.reshape(-1, D_) @ W_qkv + b_qkv
    qkv = qkv.reshape(B, S_, H, 3 * Hd).transpose(0, 2, 1, 3)
    q, k, v = np.split(qkv, 3, axis=-1)
    out = np.empty((B, S_, D_), np.float32)
    for b in range(B):
        for h in range(H):
            qb, kb, vb = q[b, h], k[b, h], v[b, h]
            lg = 2 * qb @ kb.T - (qb * qb).sum(-1)[:, None] - (kb * kb).sum(-1)[None, :]
            lg -= lg.max(-1, keepdims=True)
            w = np.exp(lg)
            w /= w.sum(-1, keepdims=True)
            out[b, :, h * Hd : (h + 1) * Hd] = w @ vb
    return (out.reshape(-1, D_) @ W_o + b_o).reshape(B, S_, D_)


def make_in_maps(x, W_qkv, W_o):
    Wr = W_qkv.reshape(D, 16, 3, HD)
    xts = [np.ascontiguousarray(x[b].T) for b in range(2)]
    in_maps = []
    for c in range(NCORES):
        b, g = c // 4, c % 4
        e0 = g * EC
        hsl = slice(NH * g, NH * (g + 1))
        in_maps.append(
            {
                "xt": xts[b],
                "wq": np.ascontiguousarray(Wr[:, hsl, 0, :].reshape(D, EC)),
                "wk": np.ascontiguousarray(Wr[:, hsl, 1, :].reshape(D, EC)),
                "wv": np.ascontiguousarray(Wr[:, hsl, 2, :].reshape(D, EC)),
                "wo": np.ascontiguousarray(W_o[e0 : e0 + EC, :]),
            }
        )
    return in_maps


def kernel(x, W_qkv, b_qkv, W_o, b_o):
    x = np.ascontiguousarray(np.asarray(x, dtype=np.float32))
    W_qkv = np.ascontiguousarray(np.asarray(W_qkv, dtype=np.float32))
    b_qkv = np.asarray(b_qkv, dtype=np.float32)
    W_o = np.ascontiguousarray(np.asarray(W_o, dtype=np.float32))
    b_o = np.asarray(b_o, dtype=np.float32)

    if np.any(b_qkv):
        return _numpy_fallback(x, W_qkv, b_qkv, W_o, b_o)

    if "nc" not in _CACHED:
        _CACHED["nc"] = build_program()
    nc = _CACHED["nc"]

    in_maps = make_in_maps(x, W_qkv, W_o)
    kw = {}
    if TRACE:
        kw = dict(trace=True, trace_cores=list(range(NCORES)))
    res = run_bass_kernel_spmd(nc, in_maps, core_ids=list(range(NCORES)), **kw)
    global LAST_RESULT
    LAST_RESULT = res

    out = np.zeros((2, S, D), np.float32)
    for c in range(NCORES):
        out[c // 4] += res.results[c]["y"].astype(np.float32)
    out += b_o
    return out
